# revision 8
# baseline (speedup 1.0000x reference)
"""Trainium2 Bass kernel for a DP-GAT layer (dense masked attention).

Computes, for x:[B,N,D], A_shape:[N,N] (0/1 adjacency), q,k,v:[D,D]:
    Q = x@q ; K = x@k
    S = Q @ K^T / sqrt(D)
    W = exp(8*tanh(S/8)) * A_shape
    out = (W / W.sum(-1, keepdims=True)) @ x @ v

Sharding: rows of N split across 8 NeuronCores (1024 rows each), SPMD,
no collectives. Each core streams its row-block of the mask, computes
scores in a flash-attention-style fused loop, and writes its row-block
of the output. Host scatters inputs / gathers outputs.

Device-side math layout (per core, per batch):
    KT  = k^T @ x^T            [D, N]    (fp32r matmuls, x^T provided by host)
    QT  = q^T @ xrows^T        [D, RB]
    xv  = x @ v (+ ones col)   [N, D+1]  fp16
    per i-chunk of 512 query rows:
      per group of 4 key-tiles (512 keys):
        S^T  = KT_tile^T @ QT_chunk      -> PSUM [128, 4, 512] (fp32r)
        u    = tanh(S^T / (8*sqrt(D)))   -> SBUF fp32   (ScalarE, scale fused)
        w    = exp(8*u)                  -> SBUF fp16   (ScalarE, scale fused)
        p    = w * maskT_tile            -> SBUF fp16   (VectorE)
        acc[i,0:129] += p_slice^T @ xv   -> PSUM        (fp16 matmuls; col 128
                                                         accumulates the rowsum
                                                         via the ones column)
      out = acc[:, :128] * (1/acc[:, 128])  -> DMA to DRAM
"""

import math
import sys
from contextlib import ExitStack

import numpy as np

try:
    import concourse.bass as bass  # noqa: F401
except ImportError:  # pragma: no cover
    sys.path.insert(0, "/opt/trn_rl_repo")
    import concourse.bass as bass  # noqa: F401

import concourse.mybir as mybir
import concourse.tile as tile
from concourse import bacc
from concourse.bass_utils import run_bass_kernel_spmd

F32 = mybir.dt.float32
F32R = mybir.dt.float32r
F16 = mybir.dt.float16

B, N, D = 4, 8192, 128
NCORES = 8
RB = N // NCORES  # query rows per core

IC = 512          # query-row chunk (free dim of score matmuls)
NIC = RB // IC    # i-chunks per core
JG = 4            # key 128-tiles per score group
NJT = N // 128    # key tiles total
NG = NJT // JG    # groups per i-chunk
CH = 512          # xt prefetch chunk width


def build_program():
    nc = bacc.Bacc("TRN2", target_bir_lowering=False, debug=False)

    xt = nc.dram_tensor("xt", [B, D, N], F32, kind="ExternalInput").ap()
    xqt = nc.dram_tensor("xqt", [B, D, RB], F32, kind="ExternalInput").ap()
    maskT = nc.dram_tensor("maskT", [N, RB], F16, kind="ExternalInput").ap()
    q_d = nc.dram_tensor("q", [D, D], F32, kind="ExternalInput").ap()
    k_d = nc.dram_tensor("k", [D, D], F32, kind="ExternalInput").ap()
    v_d = nc.dram_tensor("v", [D, D], F32, kind="ExternalInput").ap()
    out_d = nc.dram_tensor("out", [B, RB, D], F32, kind="ExternalOutput").ap()

    # [128, key-tile, query-col] view of the transposed mask block
    maskT_r = maskT.rearrange("(t p) i -> p t i", p=128)

    tanh_scale = 1.0 / (8.0 * math.sqrt(float(D)))

    with tile.TileContext(nc) as tc, ExitStack() as ctx:
        consts = ctx.enter_context(tc.tile_pool(name="consts", bufs=1))
        kt_pool = ctx.enter_context(tc.tile_pool(name="kt", bufs=2))
        qt_pool = ctx.enter_context(tc.tile_pool(name="qt", bufs=2))
        xv_pool = ctx.enter_context(tc.tile_pool(name="xv", bufs=2))
        xc_pool = ctx.enter_context(tc.tile_pool(name="xc", bufs=3))
        m_pool = ctx.enter_context(tc.tile_pool(name="m", bufs=3))
        u_pool = ctx.enter_context(tc.tile_pool(name="u", bufs=2))
        w_pool = ctx.enter_context(tc.tile_pool(name="w", bufs=2))
        p_pool = ctx.enter_context(tc.tile_pool(name="p", bufs=2))
        ob_pool = ctx.enter_context(tc.tile_pool(name="ob", bufs=4))
        rs_pool = ctx.enter_context(tc.tile_pool(name="rs", bufs=4))
        prep_ps = ctx.enter_context(tc.tile_pool(name="prep_ps", bufs=2, space="PSUM"))
        st_ps = ctx.enter_context(tc.tile_pool(name="st_ps", bufs=1, space="PSUM"))
        acc_ps = ctx.enter_context(tc.tile_pool(name="acc_ps", bufs=1, space="PSUM"))

        zeros = consts.tile([128, 512], F16)
        nc.vector.memset(zeros[:], 0.0)

        q_sb = consts.tile([D, D], F32)
        nc.sync.dma_start(q_sb[:], q_d[:])
        k_sb = consts.tile([D, D], F32)
        nc.sync.dma_start(k_sb[:], k_d[:])
        v_sb = consts.tile([D, D], F32)
        nc.sync.dma_start(v_sb[:], v_d[:])

        for b in range(B):
            # ---- per-batch prep: KT [D, N], QT [D, RB], xv [N-tiles, 130] ----
            kt = kt_pool.tile([128, N], F16)
            qt = qt_pool.tile([128, RB], F16)
            xv = xv_pool.tile([128, NJT, 130], F16)
            nc.vector.memset(xv[:, :, 128:129], 1.0)

            xq = qt_pool.tile([128, RB], F32, tag="xq")
            nc.sync.dma_start(xq[:], xqt[b])
            qch = min(CH, RB)
            for c in range(RB // qch):
                pq = prep_ps.tile([128, qch], F32, tag="prep")
                nc.tensor.matmul(
                    pq[:],
                    q_sb[:],
                    xq[:, c * qch : (c + 1) * qch],
                    start=True,
                    stop=True,
                )
                nc.vector.tensor_copy(qt[:, c * qch : (c + 1) * qch], pq[:])

            for c in range(N // CH):
                xc = xc_pool.tile([128, CH], F32)
                nc.sync.dma_start(xc[:], xt[b][:, c * CH : (c + 1) * CH])
                pk = prep_ps.tile([128, CH], F32, tag="prep")
                nc.tensor.matmul(
                    pk[:],
                    k_sb[:],
                    xc[:],
                    start=True,
                    stop=True,
                )
                nc.vector.tensor_copy(kt[:, c * CH : (c + 1) * CH], pk[:])
                for s in range(CH // 128):
                    pxv = prep_ps.tile([128, 128], F32, tag="prep")
                    nc.tensor.matmul(
                        pxv[:],
                        xc[:, s * 128 : (s + 1) * 128],
                        v_sb[:],
                        start=True,
                        stop=True,
                    )
                    nc.vector.tensor_copy(
                        xv[:, c * (CH // 128) + s, 0:128], pxv[:]
                    )

            # ---- main fused attention loop ----
            for ic in range(NIC):
                acc = acc_ps.tile([128, 1024], F32)
                # PE start=True clears the WHOLE PSUM bank, so slots that
                # share a bank must not each issue start=True (the second
                # would wipe the first's data). Clear each bank once with a
                # full-bank zero matmul; all real PV matmuls accumulate.
                for hb in range(2):
                    nc.tensor.matmul(
                        acc[:, hb * 512 : (hb + 1) * 512],
                        zeros[:, 0:128],
                        zeros[:],
                        start=True,
                        stop=False,
                        skip_group_check=True,
                    )
                for g in range(NG):
                    stp = st_ps.tile([128, JG, IC], F32)
                    for j in range(JG):
                        nc.tensor.matmul(
                            stp[:, j],
                            kt[:, (g * JG + j) * 128 : (g * JG + j + 1) * 128],
                            qt[:, ic * IC : (ic + 1) * IC],
                            start=True,
                            stop=True,
                        )
                    u = u_pool.tile([128, JG, IC], F32)
                    nc.scalar.activation(
                        u[:],
                        stp[:],
                        mybir.ActivationFunctionType.Tanh,
                        scale=tanh_scale,
                    )
                    w = w_pool.tile([128, JG, IC], F16)
                    nc.scalar.activation(
                        w[:], u[:], mybir.ActivationFunctionType.Exp, scale=8.0
                    )
                    m = m_pool.tile([128, JG, IC], F16)
                    nc.sync.dma_start(
                        m[:],
                        maskT_r[:, g * JG : (g + 1) * JG, ic * IC : (ic + 1) * IC],
                    )
                    p = p_pool.tile([128, JG, IC], F16)
                    nc.vector.tensor_mul(p[:], w[:], m[:])
                    for j in range(JG):
                        for s in range(IC // 128):
                            nc.tensor.matmul(
                                acc[:, s * 256 : s * 256 + 129],
                                p[:, j, s * 128 : (s + 1) * 128],
                                xv[:, g * JG + j, 0:129],
                                start=False,
                                stop=(g == NG - 1 and j == JG - 1),
                                skip_group_check=True,
                            )
                for s in range(IC // 128):
                    rs = rs_pool.tile([128, 1], F32)
                    nc.vector.reciprocal(rs[:], acc[:, s * 256 + 128 : s * 256 + 129])
                    ob = ob_pool.tile([128, 128], F32)
                    nc.vector.tensor_scalar_mul(ob[:], acc[:, s * 256 : s * 256 + 128], rs[:])
                    nc.sync.dma_start(
                        out_d[b, ic * IC + s * 128 : ic * IC + (s + 1) * 128, :],
                        ob[:],
                    )

    nc.compile()
    return nc


_CACHED_NC = None


def _get_program():
    global _CACHED_NC
    if _CACHED_NC is None:
        _CACHED_NC = build_program()
    return _CACHED_NC


def make_in_maps(x, A_shape, q, k, v):
    x = np.ascontiguousarray(x, dtype=np.float32)
    xt = np.ascontiguousarray(x.transpose(0, 2, 1))  # [B, D, N]
    q = np.ascontiguousarray(q, dtype=np.float32)
    k = np.ascontiguousarray(k, dtype=np.float32)
    v = np.ascontiguousarray(v, dtype=np.float32)
    in_maps = []
    for c in range(NCORES):
        r0 = c * RB
        xqt = np.ascontiguousarray(x[:, r0 : r0 + RB, :].transpose(0, 2, 1))
        maskT = np.ascontiguousarray(
            A_shape[r0 : r0 + RB, :].T, dtype=np.float16
        )
        in_maps.append(
            {"xt": xt, "xqt": xqt, "maskT": maskT, "q": q, "k": k, "v": v}
        )
    return in_maps


def kernel(x, A_shape, q, k, v):
    nc = _get_program()
    in_maps = make_in_maps(x, A_shape, q, k, v)
    res = run_bass_kernel_spmd(nc, in_maps, list(range(NCORES)))
    out = np.concatenate([res.results[c]["out"] for c in range(NCORES)], axis=1)
    return out.astype(np.float32)
